# revision 3
# baseline (speedup 1.0000x reference)
"""Cost-volume correlation kernel for Trainium2 (8 NeuronCores, SPMD over batch).

Problem: corr[b, s, y, x] = mean_c in1[b,c,y,x] * in2pad[b,c,y+dy,x+dx]
with s = dy*9+dx, dy,dx in [0,9) (max displacement 4), B=8, C=128, H=W=128.

Strategy: one batch element per core. Per core, the channel contraction is
done on the TensorEngine as banded Gram matmuls: for each 8x16 tile of in1
pixels (stationary [C=128, 128 pixels]), stream the corresponding 16x24
padded-in2 neighborhood ([C=128, 384]) to produce all pairwise dots
[128 pixels, 384 region pixels] in PSUM. The 81 displacement values per
pixel are a diagonal band of that Gram tile; the band is gathered on the
host (a per-partition-offset gather is not expressible as a single on-chip
access pattern).
"""

import sys

sys.path.insert(0, "/opt/trn_rl_repo")

import numpy as np

import concourse.bass as bass  # noqa: F401
import concourse.tile as tile
from concourse import bacc, mybir
from concourse.bass_utils import run_bass_kernel_spmd

B, C, H, W = 8, 128, 128, 128
D = 4            # max displacement
WIN = 2 * D + 1  # 9
S = WIN * WIN    # 81 displacements
TY, TX = 8, 16   # in1 pixel tile
NTY, NTX = H // TY, W // TX  # 16 x 8 = 128 tiles
NT = NTY * NTX
RY, RX = TY + 2 * D, TX + 2 * D  # 16 x 24 in2 region
N = RY * RX  # 384 moving columns per matmul
HP, WP = H + 2 * D, W + 2 * D    # 136 x 136 padded in2
CHUNK = 4  # gram tiles per output DMA

LAST_RESULT = None
_NC = None


def _build():
    nc = bacc.Bacc("TRN2", target_bir_lowering=False, debug=False, num_devices=8)
    in1 = nc.dram_tensor("in1", [C, H, W], mybir.dt.float32, kind="ExternalInput")
    in2 = nc.dram_tensor("in2", [C, H, W], mybir.dt.float32, kind="ExternalInput")
    gram = nc.dram_tensor(
        "gram", [NT // CHUNK, 128, CHUNK * N], mybir.dt.float32, kind="ExternalOutput"
    )

    f32r = mybir.dt.float32r
    with tile.TileContext(nc) as tc:
        with (
            tc.tile_pool(name="big", bufs=1) as big,
            tc.tile_pool(name="ps", bufs=6, space="PSUM") as ps,
            tc.tile_pool(name="stage", bufs=3) as stage,
            tc.tile_pool(name="wpool", bufs=3) as wpool,
        ):
            a = big.tile([C, H * W], mybir.dt.float32)
            b2 = big.tile([C, HP * WP], mybir.dt.float32)
            av = a.rearrange("p (y x) -> p y x", x=W)
            b2v = b2.rearrange("p (y x) -> p y x", x=WP)

            # zero the 4-wide pad border of in2
            nc.gpsimd.memset(b2v[:, 0:D, :], 0.0)
            nc.gpsimd.memset(b2v[:, HP - D : HP, :], 0.0)
            nc.gpsimd.memset(b2v[:, D : HP - D, 0:D], 0.0)
            nc.gpsimd.memset(b2v[:, D : HP - D, WP - D : WP], 0.0)

            nc.sync.dma_start(out=av[:], in_=in1.ap())
            nc.sync.dma_start(out=b2v[:, D : HP - D, D : WP - D], in_=in2.ap())

            for ci in range(NT // CHUNK):
                g = stage.tile([128, CHUNK * N], mybir.dt.float32)
                for k in range(CHUNK):
                    t = ci * CHUNK + k
                    tyi, txi = divmod(t, NTX)
                    y0, x0 = tyi * TY, txi * TX
                    # stationary operand must be a flat contiguous AP: stage it
                    w = wpool.tile([128, 128], mybir.dt.float32)
                    nc.scalar.copy(out=w[:], in_=av[:, y0 : y0 + TY, x0 : x0 + TX])
                    psum = ps.tile([128, N], mybir.dt.float32)
                    nc.tensor.matmul(
                        psum[:],
                        w[:],
                        b2v[:, y0 : y0 + RY, x0 : x0 + RX],
                        start=True,
                        stop=True,
                    )
                    nc.vector.tensor_copy(out=g[:, k * N : (k + 1) * N], in_=psum[:])
                nc.sync.dma_start(out=gram.ap()[ci], in_=g[:])
    nc.finalize()
    return nc


def _get_nc():
    global _NC
    if _NC is None:
        _NC = _build()
    return _NC


# host-side band extraction indices: for pixel p=(py,px) and shift s=(dy,dx),
# gram column is (py+dy)*RX + (px+dx)
_p = np.arange(128)
_py, _px = _p // TX, _p % TX
_s = np.arange(S)
_dy, _dx = _s // WIN, _s % WIN
_COL = (_py[:, None] + _dy[None, :]) * RX + (_px[:, None] + _dx[None, :])  # [128, 81]


def kernel(in1, in2):
    global LAST_RESULT
    in1 = np.asarray(in1, dtype=np.float32)
    in2 = np.asarray(in2, dtype=np.float32)
    nc = _get_nc()
    in_maps = [
        {"in1": np.ascontiguousarray(in1[b]), "in2": np.ascontiguousarray(in2[b])}
        for b in range(B)
    ]
    res = run_bass_kernel_spmd(nc, in_maps, core_ids=list(range(B)))
    LAST_RESULT = res

    out = np.empty((B, S, H, W), dtype=np.float32)
    for b in range(B):
        gram = res.results[b]["gram"]  # [NT/CHUNK, 128, CHUNK*N]
        gram = (
            gram.reshape(NT // CHUNK, 128, CHUNK, N)
            .transpose(0, 2, 1, 3)
            .reshape(NT, 128, N)
        )
        ext = gram[:, _p[:, None], _COL]  # [NT, 128, 81]
        # tile t=(tyi,txi), pixel p=(py,px) -> y=tyi*TY+py, x=txi*TX+px
        ext = ext.reshape(NTY, NTX, TY, TX, S)
        out[b] = (
            ext.transpose(4, 0, 2, 1, 3).reshape(S, H, W) * np.float32(1.0 / C)
        )
    return out


# revision 5
# speedup vs baseline: 1.7787x; 1.7787x over previous
"""Cost-volume correlation kernel for Trainium2 (8 NeuronCores, SPMD over batch).

Problem: corr[b, s, y, x] = mean_c in1[b,c,y,x] * in2pad[b,c,y+dy,x+dx]
with s = dy*9+dx, dy,dx in [0,9) (max displacement 4), B=8, C=128, H=W=128.

Strategy: one batch element per core (data-parallel over B). Per core, the
channel contraction runs on the TensorEngine as banded Gram matmuls: for
each 8x16 tile of in1 pixels (stationary [C=128, 128 pixels], bf16), stream
the 16x24 padded-in2 neighborhood ([C=128, 384], bf16) to produce all
pairwise dots [128 pixels, 384 region pixels] in PSUM (fp32). The 81
displacement values per pixel form a diagonal band of the Gram tile; a
per-partition-offset gather is not expressible on-chip (verified: DMA AP
dim0 is partitions-only, compute APs are partition-uniform), so Gram tiles
are DMA'd out in bf16 and the band is gathered host-side.

Host-side prep per core: in1 is pre-tiled (so each tile's stationary
operand is a flat contiguous SBUF slice — matmul weights reject multi-dim
APs), in2 is pre-padded, and both are pre-cast to bf16, halving input DMA
traffic.
"""

import sys

sys.path.insert(0, "/opt/trn_rl_repo")

import ml_dtypes
import numpy as np

import concourse.bass as bass  # noqa: F401
import concourse.tile as tile
from concourse import bacc, mybir
from concourse.bass_utils import run_bass_kernel_spmd

B, C, H, W = 8, 128, 128, 128
D = 4            # max displacement
WIN = 2 * D + 1  # 9
S = WIN * WIN    # 81 displacements
TY, TX = 8, 16   # in1 pixel tile
NTY, NTX = H // TY, W // TX  # 16 x 8 = 128 tiles
NT = NTY * NTX
RY, RX = TY + 2 * D, TX + 2 * D  # 16 x 24 in2 region
N = RY * RX  # 384 moving columns per matmul
HP, WP = H + 2 * D, W + 2 * D    # 136 x 136 padded in2
CHUNK = 8  # gram tiles per output DMA

LAST_RESULT = None
_NC = None

BF16 = mybir.dt.bfloat16
F32 = mybir.dt.float32
NP_BF16 = ml_dtypes.bfloat16


def _build():
    nc = bacc.Bacc("TRN2", target_bir_lowering=False, debug=False, num_devices=8)
    # in1t: host-pre-tiled bf16 in1, [C, NT*128], tile t at cols [t*128,(t+1)*128)
    in1t = nc.dram_tensor("in1t", [C, NT * TY * TX], BF16, kind="ExternalInput")
    # in2p: host-pre-padded bf16 in2, [C, HP*WP]
    in2p = nc.dram_tensor("in2p", [C, HP * WP], BF16, kind="ExternalInput")
    gram = nc.dram_tensor(
        "gram", [NT // CHUNK, 128, CHUNK * N], BF16, kind="ExternalOutput"
    )

    with tile.TileContext(nc) as tc:
        with (
            tc.tile_pool(name="big", bufs=1) as big,
            tc.tile_pool(name="ps", bufs=6, space="PSUM") as ps,
            tc.tile_pool(name="stage", bufs=3) as stage,
        ):
            a = big.tile([C, NT * TY * TX], BF16)
            b2 = big.tile([C, HP * WP], BF16)
            b2v = b2.rearrange("p (y x) -> p y x", x=WP)

            nc.sync.dma_start(out=a[:], in_=in1t.ap())
            nc.sync.dma_start(out=b2[:], in_=in2p.ap())

            for ci in range(NT // CHUNK):
                g = stage.tile([128, CHUNK * N], BF16)
                for k in range(CHUNK):
                    t = ci * CHUNK + k
                    tyi, txi = divmod(t, NTX)
                    y0, x0 = tyi * TY, txi * TX
                    psum = ps.tile([128, N], F32)
                    nc.tensor.matmul(
                        psum[:],
                        a[:, t * 128 : (t + 1) * 128],
                        b2v[:, y0 : y0 + RY, x0 : x0 + RX],
                        start=True,
                        stop=True,
                    )
                    # PSUM->SBUF copy (casts fp32->bf16); balance DVE/ACT 2:1
                    if k % 3 == 2:
                        nc.scalar.copy(out=g[:, k * N : (k + 1) * N], in_=psum[:])
                    else:
                        nc.vector.tensor_copy(
                            out=g[:, k * N : (k + 1) * N], in_=psum[:]
                        )
                nc.sync.dma_start(out=gram.ap()[ci], in_=g[:])
    nc.finalize()
    return nc


def _get_nc():
    global _NC
    if _NC is None:
        _NC = _build()
    return _NC


# host-side band extraction indices: for pixel p=(py,px) and shift s=(dy,dx),
# gram column is (py+dy)*RX + (px+dx)
_p = np.arange(128)
_py, _px = _p // TX, _p % TX
_s = np.arange(S)
_dy, _dx = _s // WIN, _s % WIN
_COL = (_py[:, None] + _dy[None, :]) * RX + (_px[:, None] + _dx[None, :])  # [128, 81]


def _pretile_in1(x):  # [C, H, W] fp32 -> [C, NT*128] bf16 tile-contiguous
    return np.ascontiguousarray(
        x.reshape(C, NTY, TY, NTX, TX)
        .transpose(0, 1, 3, 2, 4)
        .reshape(C, NT * TY * TX)
        .astype(NP_BF16)
    )


def _prepad_in2(x):  # [C, H, W] fp32 -> [C, HP*WP] bf16 zero-padded
    p = np.zeros((C, HP, WP), dtype=NP_BF16)
    p[:, D : D + H, D : D + W] = x.astype(NP_BF16)
    return p.reshape(C, HP * WP)


def kernel(in1, in2):
    global LAST_RESULT
    in1 = np.asarray(in1, dtype=np.float32)
    in2 = np.asarray(in2, dtype=np.float32)
    nc = _get_nc()
    in_maps = [
        {"in1t": _pretile_in1(in1[b]), "in2p": _prepad_in2(in2[b])} for b in range(B)
    ]
    res = run_bass_kernel_spmd(nc, in_maps, core_ids=list(range(B)))
    LAST_RESULT = res

    out = np.empty((B, S, H, W), dtype=np.float32)
    for b in range(B):
        gram = res.results[b]["gram"]  # [NT/CHUNK, 128, CHUNK*N] bf16
        gram = (
            np.asarray(gram)
            .reshape(NT // CHUNK, 128, CHUNK, N)
            .transpose(0, 2, 1, 3)
            .reshape(NT, 128, N)
            .astype(np.float32)
        )
        ext = gram[:, _p[:, None], _COL]  # [NT, 128, 81]
        ext = ext.reshape(NTY, NTX, TY, TX, S)
        out[b] = ext.transpose(4, 0, 2, 1, 3).reshape(S, H, W) * np.float32(1.0 / C)
    return out


# revision 8
# speedup vs baseline: 2.3002x; 1.2932x over previous
"""Cost-volume correlation kernel for Trainium2 (8 NeuronCores, SPMD over batch).

Problem: corr[b, s, y, x] = mean_c in1[b,c,y,x] * in2pad[b,c,y+dy,x+dx]
with s = dy*9+dx, dy,dx in [0,9) (max displacement 4), B=8, C=128, H=W=128.

Strategy: one batch element per core (data-parallel over B). Per core, the
channel contraction runs on the TensorEngine as banded Gram matmuls: for
each 8x16 tile of in1 pixels (stationary [C=128, 128 pixels], bf16), stream
the 16x24 padded-in2 neighborhood ([C=128, 384], bf16) to produce all
pairwise dots [128 pixels, 384 region pixels] in PSUM (fp32). The 81
displacement values per pixel form a diagonal band of the Gram tile; a
per-partition-offset gather is not expressible on-chip (verified: DMA AP
dim0 is partitions-only, compute APs are partition-uniform), so Gram tiles
are DMA'd out in bf16 and the band is gathered host-side.

Host-side prep per core: in1 is pre-tiled (so each tile's stationary
operand is a flat contiguous SBUF slice — matmul weights reject multi-dim
APs), in2 is pre-padded, and both are pre-cast to bf16, halving input DMA
traffic.
"""

import sys

sys.path.insert(0, "/opt/trn_rl_repo")

import ml_dtypes
import numpy as np

import concourse.bass as bass  # noqa: F401
import concourse.tile as tile
from concourse import bacc, mybir
from concourse.bass_utils import run_bass_kernel_spmd

B, C, H, W = 8, 128, 128, 128
D = 4            # max displacement
WIN = 2 * D + 1  # 9
S = WIN * WIN    # 81 displacements
TY, TX = 8, 16   # in1 pixel tile
NTY, NTX = H // TY, W // TX  # 16 x 8 = 128 tiles
NT = NTY * NTX
RY, RX = TY + 2 * D, TX + 2 * D  # 16 x 24 in2 region
N = RY * RX  # 384 moving columns per matmul
HP, WP = H + 2 * D, W + 2 * D    # 136 x 136 padded in2
CHUNK = 8  # gram tiles per output DMA

LAST_RESULT = None
_NC = None

BF16 = mybir.dt.bfloat16
F32 = mybir.dt.float32
NP_BF16 = ml_dtypes.bfloat16


def _build():
    nc = bacc.Bacc("TRN2", target_bir_lowering=False, debug=False, num_devices=8)
    # in1t: host-pre-tiled bf16 in1, [C, NT*128], tile t at cols [t*128,(t+1)*128)
    in1t = nc.dram_tensor("in1t", [C, NT * TY * TX], BF16, kind="ExternalInput")
    # in2p: host-pre-padded bf16 in2, [C, HP*WP]
    in2p = nc.dram_tensor("in2p", [C, HP * WP], BF16, kind="ExternalInput")
    gram = nc.dram_tensor(
        "gram", [NT // CHUNK, 128, CHUNK * N], BF16, kind="ExternalOutput"
    )

    with tile.TileContext(nc) as tc:
        with (
            tc.tile_pool(name="big", bufs=1) as big,
            tc.tile_pool(name="ps", bufs=8, space="PSUM") as ps,
            tc.tile_pool(name="stage", bufs=3) as stage,
        ):
            a = big.tile([C, NT * TY * TX], BF16)
            b2 = big.tile([C, HP * WP], BF16)
            b2v = b2.rearrange("p (y x) -> p y x", x=WP)

            # chunked loads so early matmuls overlap the input DMA
            in1_chunk = NT // 8 * TY * TX  # 16 tiles worth
            for j in range(8):
                nc.sync.dma_start(
                    out=a[:, j * in1_chunk : (j + 1) * in1_chunk],
                    in_=in1t.ap()[:, j * in1_chunk : (j + 1) * in1_chunk],
                )
            in2_chunk = 17 * WP  # 17 padded rows
            for j in range(8):
                nc.sync.dma_start(
                    out=b2[:, j * in2_chunk : (j + 1) * in2_chunk],
                    in_=in2p.ap()[:, j * in2_chunk : (j + 1) * in2_chunk],
                )

            for ci in range(NT // CHUNK):
                g = stage.tile([128, CHUNK * N], BF16)
                for k in range(CHUNK):
                    t = ci * CHUNK + k
                    tyi, txi = divmod(t, NTX)
                    y0, x0 = tyi * TY, txi * TX
                    psum = ps.tile([128, N], F32)
                    nc.tensor.matmul(
                        psum[:],
                        a[:, t * 128 : (t + 1) * 128],
                        b2v[:, y0 : y0 + RY, x0 : x0 + RX],
                        start=True,
                        stop=True,
                    )
                    # PSUM->SBUF copy (casts fp32->bf16); balance DVE/ACT 1:1
                    if k % 2 == 1:
                        nc.scalar.copy(out=g[:, k * N : (k + 1) * N], in_=psum[:])
                    else:
                        nc.vector.tensor_copy(
                            out=g[:, k * N : (k + 1) * N], in_=psum[:]
                        )
                nc.sync.dma_start(out=gram.ap()[ci], in_=g[:])
    nc.finalize()
    return nc


def _get_nc():
    global _NC
    if _NC is None:
        _NC = _build()
    return _NC


# host-side band extraction indices: for pixel p=(py,px) and shift s=(dy,dx),
# gram column is (py+dy)*RX + (px+dx)
_p = np.arange(128)
_py, _px = _p // TX, _p % TX
_s = np.arange(S)
_dy, _dx = _s // WIN, _s % WIN
_COL = (_py[:, None] + _dy[None, :]) * RX + (_px[:, None] + _dx[None, :])  # [128, 81]


def _pretile_in1(x):  # [C, H, W] fp32 -> [C, NT*128] bf16 tile-contiguous
    return np.ascontiguousarray(
        x.reshape(C, NTY, TY, NTX, TX)
        .transpose(0, 1, 3, 2, 4)
        .reshape(C, NT * TY * TX)
        .astype(NP_BF16)
    )


def _prepad_in2(x):  # [C, H, W] fp32 -> [C, HP*WP] bf16 zero-padded
    p = np.zeros((C, HP, WP), dtype=NP_BF16)
    p[:, D : D + H, D : D + W] = x.astype(NP_BF16)
    return p.reshape(C, HP * WP)


def kernel(in1, in2):
    global LAST_RESULT
    in1 = np.asarray(in1, dtype=np.float32)
    in2 = np.asarray(in2, dtype=np.float32)
    nc = _get_nc()
    in_maps = [
        {"in1t": _pretile_in1(in1[b]), "in2p": _prepad_in2(in2[b])} for b in range(B)
    ]
    res = run_bass_kernel_spmd(nc, in_maps, core_ids=list(range(B)))
    LAST_RESULT = res

    out = np.empty((B, S, H, W), dtype=np.float32)
    for b in range(B):
        gram = res.results[b]["gram"]  # [NT/CHUNK, 128, CHUNK*N] bf16
        gram = (
            np.asarray(gram)
            .reshape(NT // CHUNK, 128, CHUNK, N)
            .transpose(0, 2, 1, 3)
            .reshape(NT, 128, N)
            .astype(np.float32)
        )
        ext = gram[:, _p[:, None], _COL]  # [NT, 128, 81]
        ext = ext.reshape(NTY, NTX, TY, TX, S)
        out[b] = ext.transpose(4, 0, 2, 1, 3).reshape(S, H, W) * np.float32(1.0 / C)
    return out
